# Initial kernel scaffold
#
"""Multi-head attention TRN2 Bass kernel.

Problem: B=4, S=2048, D=1024, H=16 heads (DK=64), fp32, random 0/1
attention mask broadcast over heads.

Sharding: 8 cores = (batch b, query-half) pairs. Core c handles batch
c//2, query rows [ (c%2)*1024, (c%2+1)*1024 ).  K/V projections for the
batch are computed redundantly on the 2 cores sharing a batch; no
collectives are needed and each core writes a disjoint output slice.

Layout strategy (per core):
  - Host pre-transposes q, k, v (and the mask) so the feature dim lands
    on SBUF partitions; projections then run without any on-chip
    transposes.
  - qhT [D, SQ], khT [D, S] are produced transposed (out-feature on
    partitions) and spilled to DRAM; vh is produced in natural layout
    [S, D] extended with a ones column per head ([vh_h | 1], width 65).
  - scoresT_chunk[k, q] = khT_chunk.T @ qhT  (k on partitions) via
    matmul(lhsT=khT[64, 128chunk], rhs=qhT[64, 512]).
  - exp on ACT (no max subtraction: scores/8 are O(1), mask applied
    multiplicatively post-exp), mask-multiply on DVE.
  - AV: matmul(lhsT=[vh_h | 1][128, 65], rhs=mexpT[128, 512]) accumulated
    over 16 k-chunks -> PSUM [65, 1024]: rows 0..63 = unnormalized out^T,
    row 64 = softmax denominators.
  - normalize: DMA-broadcast denominators to 64 partitions (via a DRAM
    round-trip: SBUF-source DMAs cannot have a zero partition step), DVE
    reciprocal + multiply; result IS the transposed lhsT for the output
    projection.

All matmuls run as float32r (TF32-like rounding, 1 row/cycle vs 4 for
fp32; bit-identical storage).  Measured on 8 axon trn2 cores:
relative error 1.6e-04 vs the fp32 jax reference, ~1.07 ms/exec
(marginal cost of an extra pipelined NEFF execution, all I/O in DRAM).
"""

import os
import sys

if "/opt/trn_rl_repo" not in sys.path:
    sys.path.insert(0, "/opt/trn_rl_repo")
os.environ.setdefault("MYCRO_LOCAL_CACHE", "1")

import numpy as np
import ml_dtypes

import concourse.bass as bass
import concourse.bacc as bacc
import concourse.mybir as mybir
import concourse.tile as tile
from concourse.bass import ts

B, S, D, H, DK = 4, 2048, 1024, 16, 64
SQ = S // 2          # q rows per core
P = 128
NCHUNK = S // P      # 16 k-chunks
NJ = D // P          # 8 feature chunks
NQT = SQ // P        # 8 q tiles
N_CORES = 8

F32 = mybir.dt.float32
BF16 = mybir.dt.bfloat16
AF = mybir.ActivationFunctionType

# matmul compute dtype: float32r streams 1 row/cycle (vs 4 for float32)
# at N>=256.  Bits are identical to fp32; precision measured on HW.
# MD is the dtype of every tensor that feeds a matmul operand.  float32r
# is required to be plumbed through producer output dtypes (walrus BIR
# verifier: "consumed by FP32r matmult but is not rounded to FP32r").
FAST = os.environ.get("MHA_MM_DT", "f32r") == "f32r"
MD = mybir.dt.float32r if FAST else mybir.dt.float32


def _mm(ap):
    return ap


def build_program(n_iters=1):
    nc = bacc.Bacc(
        "TRN2",
        target_bir_lowering=False,
        debug=False,
        enable_asserts=False,
    )

    # ---- DRAM I/O (per-core slices; host pre-transposed) ----
    qT_d = nc.dram_tensor("qT", [D, SQ], MD, kind="ExternalInput").ap()
    kT_d = nc.dram_tensor("kT", [D, S], MD, kind="ExternalInput").ap()
    vT_d = nc.dram_tensor("vT", [D, S], MD, kind="ExternalInput").ap()
    mT_d = nc.dram_tensor("maskT", [S, SQ], BF16, kind="ExternalInput").ap()
    wq_d = nc.dram_tensor("wq", [D, D], MD, kind="ExternalInput").ap()
    wk_d = nc.dram_tensor("wk", [D, D], MD, kind="ExternalInput").ap()
    wv_d = nc.dram_tensor("wv", [D, D], MD, kind="ExternalInput").ap()
    wo_d = nc.dram_tensor("wo", [D, D], MD, kind="ExternalInput").ap()
    bq_d = nc.dram_tensor("bq", [D], F32, kind="ExternalInput").ap()
    bk_d = nc.dram_tensor("bk", [D], F32, kind="ExternalInput").ap()
    bv_d = nc.dram_tensor("bv", [D], MD, kind="ExternalInput").ap()
    bo_d = nc.dram_tensor("bo", [D], MD, kind="ExternalInput").ap()
    ones_d = nc.dram_tensor("ones_row", [1, P], MD, kind="ExternalInput").ap()
    ones16_d = nc.dram_tensor("ones16", [P, H], MD, kind="ExternalInput").ap()
    out_d = nc.dram_tensor("out", [SQ, D], F32, kind="ExternalOutput").ap()

    with tile.TileContext(nc) as tc:
        for _ in range(n_iters):
            _build(nc, tc, qT_d, kT_d, vT_d, mT_d,
                   wq_d, wk_d, wv_d, wo_d, bq_d, bk_d, bv_d, bo_d, out_d,
                   ones_d, ones16_d)

    nc.compile()
    return nc


def _proj_T(nc, tc, ctx, x_d, w_d, bias_col, out_sink, scale, n_src, tagp=""):
    """Transposed projection: out[j, r] = sum_d w[d, j] * xT[d, r] + b[j].

    x_d: DRAM [D, n_src] (input, pre-transposed).  w_d: DRAM [D, D].
    bias_col: SBUF [P, NJ] per-feature bias columns (already scaled).
    out_sink(j, half, stage_ap): consume the [P, SQ] result slab for
    output features [j*128, (j+1)*128) and source rows
    [half*SQ, half*SQ+SQ).
    """
    nhalf = n_src // SQ
    win = ctx.enter_context(tc.tile_pool(name=f"win{tagp}", bufs=1))
    xT = [win.tile([P, SQ], MD, tag=f"xT{tagp}{d}", name=f"xT{tagp}{d}")
          for d in range(NJ)]
    wti = [win.tile([P, D], MD, tag=f"wti{tagp}{d}", name=f"wti{tagp}{d}")
           for d in range(NJ)]
    stage = ctx.enter_context(tc.tile_pool(name=f"stage{tagp}", bufs=3))
    psum = ctx.enter_context(tc.tile_pool(name=f"ps{tagp}", bufs=2, space="PSUM"))
    for d in range(NJ):
        nc.sync.dma_start(wti[d][:], w_d[ts(d, P), :])
    for half in range(nhalf):
        for d in range(NJ):
            nc.sync.dma_start(xT[d][:], x_d[ts(d, P), ts(half, SQ)])
        for j in range(NJ):
            pq = psum.tile([P, SQ], F32, tag="pq")
            for d in range(NJ):
                for h2 in range(SQ // 512):
                    nc.tensor.matmul(
                        pq[:, ts(h2, 512)],
                        _mm(wti[d][:, ts(j, P)]),
                        _mm(xT[d][:, ts(h2, 512)]),
                        start=(d == 0), stop=(d == NJ - 1),
                    )
            st = stage.tile([P, SQ], MD, tag="stp")
            nc.scalar.activation(st[:], pq[:], AF.Identity,
                                 bias=bias_col[:, ts(j, 1)], scale=scale)
            out_sink(j, half, st)


def _build(nc, tc, qT_d, kT_d, vT_d, mT_d,
           wq_d, wk_d, wv_d, wo_d, bq_d, bk_d, bv_d, bo_d, out_d,
           ones_d, ones16_d):
    from contextlib import ExitStack

    with ExitStack() as top:
        dram = top.enter_context(tc.tile_pool(name="dram", bufs=1, space="DRAM"))
        qhT_dram = dram.tile([NJ, P, SQ], MD)   # q-head projections, transposed
        khT_dram = dram.tile([NJ, P, S], MD)    # k-head projections, transposed
        ctT_dram = dram.tile([NJ, P, SQ], MD)   # normalized attention out^T
        sums_dram = dram.tile([H, SQ], F32)      # per-head softmax denominators

        consts = top.enter_context(tc.tile_pool(name="consts", bufs=1))
        ones_row = consts.tile([1, P], MD, tag="ones_row")
        nc.sync.dma_start(ones_row[:], ones_d)
        bv_row = consts.tile([1, D], MD, tag="bv_row")
        nc.sync.dma_start(bv_row[:], bv_d.rearrange("(o n) -> o n", o=1))
        bo_row = consts.tile([1, D], MD, tag="bo_row")
        nc.sync.dma_start(bo_row[:], bo_d.rearrange("(o n) -> o n", o=1))
        # per-chunk bias columns [128, 1]
        bq_c = consts.tile([P, NJ], F32, tag="bq_c")
        nc.sync.dma_start(bq_c[:], bq_d.rearrange("(j p) -> p j", p=P))
        nc.vector.tensor_scalar_mul(bq_c[:], bq_c[:], 0.125)  # fold 1/sqrt(DK)
        bk_c = consts.tile([P, NJ], F32, tag="bk_c")
        nc.sync.dma_start(bk_c[:], bk_d.rearrange("(j p) -> p j", p=P))

        # W_o preloaded here so phase G never stalls on weight DMA.
        wo_pool = top.enter_context(tc.tile_pool(name="wo", bufs=1))
        wo_t = [wo_pool.tile([P, D], MD, tag=f"wo{d}", name=f"wo{d}")
                for d in range(NJ)]
        for d in range(NJ):
            nc.sync.dma_start(wo_t[d][:], wo_d[ts(d, P), :])

        # ---------- Phases B+C: q and k projections (distinct pools so
        # C's DMA loads overlap B's matmuls) ----------
        with ExitStack() as ctx:
            def q_sink(j, half, st):
                nc.sync.dma_start(qhT_dram[j], st[:])
            _proj_T(nc, tc, ctx, qT_d, wq_d, bq_c, q_sink, 0.125, SQ, tagp="q")

            def k_sink(j, half, st):
                nc.sync.dma_start(khT_dram[j][:, ts(half, SQ)], st[:])
            _proj_T(nc, tc, ctx, kT_d, wk_d, bk_c, k_sink, 1.0, S, tagp="k")

        # persistent attention operands (allocated after B/C pools closed)
        kv_pool = top.enter_context(tc.tile_pool(name="kv", bufs=1))
        vh_ext = [kv_pool.tile([P, H * (DK + 1)], MD, tag=f"vhe{c}",
                               name=f"vhe{c}") for c in range(NCHUNK)]
        maskT = [kv_pool.tile([P, SQ], BF16, tag=f"mT{c}", name=f"mT{c}")
                 for c in range(NCHUNK)]

        # ---------- Phase D: v projection -> vh_ext (resident) ----------
        with ExitStack() as ctx:
            win = ctx.enter_context(tc.tile_pool(name="win", bufs=1))
            vT = [win.tile([P, SQ], MD, tag=f"vT{d}", name=f"vT{d}")
                  for d in range(NJ)]
            wti = [win.tile([P, D], MD, tag=f"wti{d}", name=f"wti{d}")
                   for d in range(NJ)]
            psum = ctx.enter_context(tc.tile_pool(name="psD", bufs=2, space="PSUM"))
            for d in range(NJ):
                nc.sync.dma_start(wti[d][:], wv_d[ts(d, P), :])
            for vhalf in range(2):
                for d in range(NJ):
                    nc.sync.dma_start(vT[d][:], vT_d[ts(d, P), ts(vhalf, SQ)])
                for cl in range(NCHUNK // 2):
                    c = vhalf * (NCHUNK // 2) + cl
                    nc.sync.dma_start(
                        vh_ext[c].rearrange("p (h w) -> p h w", w=DK + 1)[:, :, DK:DK + 1],
                        ones16_d.rearrange("p (h o) -> p h o", o=1),
                    )
                    pv = psum.tile([P, D], F32, tag="pv")
                    for d in range(NJ):
                        for h2 in range(D // 512):
                            nc.tensor.matmul(
                                pv[:, ts(h2, 512)],
                                _mm(vT[d][:, ts(cl, P)]),
                                _mm(wti[d][:, ts(h2, 512)]),
                                start=(d == 0), stop=False,
                            )
                    for h2 in range(D // 512):
                        nc.tensor.matmul(
                            pv[:, ts(h2, 512)],
                            _mm(ones_row[:]),
                            _mm(bv_row[:, ts(h2, 512)]),
                            start=False, stop=True,
                        )
                    for h in range(H):
                        nc.vector.tensor_copy(
                            vh_ext[c][:, h * (DK + 1): h * (DK + 1) + DK],
                            pv[:, ts(h, DK)],
                        )

        # ---------- Phase E: mask load (host pre-transposed bf16) ----------
        for c in range(NCHUNK):
            nc.sync.dma_start(maskT[c][:], mT_d[ts(c, P), :])

        # ---------- Phase F: attention ----------
        with ExitStack() as ctx:
            qp = ctx.enter_context(tc.tile_pool(name="qp", bufs=2))
            kp = ctx.enter_context(tc.tile_pool(name="kp", bufs=2))
            ep = ctx.enter_context(tc.tile_pool(name="ep", bufs=2))
            mp = ctx.enter_context(tc.tile_pool(name="mp", bufs=3))
            rp = ctx.enter_context(tc.tile_pool(name="rp", bufs=2))
            ps_s = ctx.enter_context(tc.tile_pool(name="ps_s", bufs=2, space="PSUM"))
            ps_a = ctx.enter_context(tc.tile_pool(name="ps_a", bufs=2, space="PSUM"))
            for hp in range(H // 2):
                qhT_pair = qp.tile([P, SQ], MD, tag="qhT_pair")
                nc.sync.dma_start(qhT_pair[:], qhT_dram[hp])
                khT_pair = kp.tile([P, S], MD, tag="khT_pair")
                nc.sync.dma_start(khT_pair[:], khT_dram[hp])
                for hh in range(2):
                    h = 2 * hp + hh
                    pa = ps_a.tile([DK + 1, SQ], F32, tag="pa")
                    for c in range(NCHUNK):
                        pscr = ps_s.tile([P, SQ], F32, tag="pscr")
                        for h2 in range(SQ // 512):
                            nc.tensor.matmul(
                                pscr[:, ts(h2, 512)],
                                _mm(khT_pair[ts(hh, DK), ts(c, P)]),
                                _mm(qhT_pair[ts(hh, DK), ts(h2, 512)]),
                                start=True, stop=True,
                            )
                        et = ep.tile([P, SQ], F32, tag="et")
                        nc.scalar.activation(et[:], pscr[:], AF.Exp)
                        mt = mp.tile([P, SQ], MD, tag="mt")
                        nc.vector.tensor_mul(mt[:], et[:], maskT[c][:])
                        for h2 in range(SQ // 512):
                            nc.tensor.matmul(
                                pa[:, ts(h2, 512)],
                                _mm(vh_ext[c][:, h * (DK + 1): (h + 1) * (DK + 1)]),
                                _mm(mt[:, ts(h2, 512)]),
                                start=(c == 0), stop=(c == NCHUNK - 1),
                            )
                    # normalize: rows 0..63 /= row 64
                    sums = rp.tile([1, SQ], F32, tag="sums")
                    nc.vector.tensor_copy(sums[:], pa[DK:DK + 1, :])
                    nc.sync.dma_start(
                        sums_dram[h].rearrange("(o n) -> o n", o=1), sums[:])
                    rb = rp.tile([DK, SQ], F32, tag="rb")
                    nc.sync.dma_start(
                        rb[:], sums_dram[h].rearrange("(o n) -> o n", o=1)
                        .to_broadcast((DK, SQ)))
                    nc.vector.reciprocal(rb[:], rb[:])
                    ot = rp.tile([DK, SQ], MD, tag="ot")
                    nc.vector.tensor_mul(ot[:], pa[0:DK, :], rb[:])
                    nc.sync.dma_start(ctT_dram[hp, ts(hh, DK), :], ot[:])

        # ---------- Phase G: output projection ----------
        with ExitStack() as ctx:
            win = ctx.enter_context(tc.tile_pool(name="winG", bufs=1))
            ctT = [win.tile([P, SQ], MD, tag=f"ctT{d}", name=f"ctT{d}")
                   for d in range(NJ)]
            wti = wo_t
            stage = ctx.enter_context(tc.tile_pool(name="stageG", bufs=2))
            psum = ctx.enter_context(tc.tile_pool(name="psG", bufs=2, space="PSUM"))
            for d in range(NJ):
                nc.sync.dma_start(ctT[d][:], ctT_dram[d])
            for qt in range(NQT):
                po = psum.tile([P, D], F32, tag="po")
                for d in range(NJ):
                    for h2 in range(D // 512):
                        nc.tensor.matmul(
                            po[:, ts(h2, 512)],
                            _mm(ctT[d][:, ts(qt, P)]),
                            _mm(wti[d][:, ts(h2, 512)]),
                            start=(d == 0), stop=False,
                        )
                for h2 in range(D // 512):
                    nc.tensor.matmul(
                        po[:, ts(h2, 512)],
                        _mm(ones_row[:]),
                        _mm(bo_row[:, ts(h2, 512)]),
                        start=False, stop=True,
                    )
                st = stage.tile([P, D], F32, tag="sto")
                nc.scalar.activation(st[:], po[:], AF.Copy)
                nc.sync.dma_start(out_d[ts(qt, P), :], st[:])


def make_in_maps(q, k, v, att_mask):
    """Build the 8 per-core input dicts from full inputs."""
    q = np.asarray(q, dtype=np.float32)
    k = np.asarray(k, dtype=np.float32)
    v = np.asarray(v, dtype=np.float32)
    att_mask = np.asarray(att_mask)
    in_maps = []
    kT_b = [np.ascontiguousarray(k[b].T) for b in range(B)]
    vT_b = [np.ascontiguousarray(v[b].T) for b in range(B)]
    for c in range(N_CORES):
        b, half = divmod(c, 2)
        qs = slice(half * SQ, (half + 1) * SQ)
        in_maps.append({
            "qT": np.ascontiguousarray(q[b, qs, :].T),
            "kT": kT_b[b],
            "vT": vT_b[b],
            "maskT": np.ascontiguousarray(
                att_mask[b, qs, :].T).astype(ml_dtypes.bfloat16),
        })
    return in_maps


_PROG = None


def _get_program():
    global _PROG
    if _PROG is None:
        _PROG = build_program()
    return _PROG


def kernel(q, k, v, att_mask, W_q, b_q, W_k, b_k, W_v, b_v, W_o, b_o,
           **_ignored):
    from concourse.bass_utils import run_bass_kernel_spmd

    nc = _get_program()
    weights = {
        "wq": np.ascontiguousarray(W_q, dtype=np.float32),
        "wk": np.ascontiguousarray(W_k, dtype=np.float32),
        "wv": np.ascontiguousarray(W_v, dtype=np.float32),
        "wo": np.ascontiguousarray(W_o, dtype=np.float32),
        "bq": np.ascontiguousarray(b_q, dtype=np.float32),
        "bk": np.ascontiguousarray(b_k, dtype=np.float32),
        "bv": np.ascontiguousarray(b_v, dtype=np.float32),
        "bo": np.ascontiguousarray(b_o, dtype=np.float32),
        "ones_row": np.ones((1, P), dtype=np.float32),
        "ones16": np.ones((P, H), dtype=np.float32),
    }
    in_maps = [dict(m, **weights) for m in make_in_maps(q, k, v, att_mask)]
    res = run_bass_kernel_spmd(nc, in_maps, core_ids=list(range(N_CORES)))
    out = np.empty((B, S, D), dtype=np.float32)
    for c in range(N_CORES):
        b, half = divmod(c, 2)
        out[b, half * SQ:(half + 1) * SQ, :] = res.results[c]["out"]
    return out



# revision 2
# speedup vs baseline: 1.4091x; 1.4091x over previous
"""Multi-head attention TRN2 Bass kernel, v2 (bf16, fully SBUF-resident).

Problem: B=4, S=2048, D=1024, H=16 heads (DK=64), fp32 reference, random
0/1 attention mask broadcast over heads.

Sharding: 8 cores = (batch b, query-half) pairs.  Core c handles batch
c//2, query rows [(c%2)*1024, (c%2+1)*1024).  K/V projections for the
batch are computed redundantly on the 2 cores sharing a batch; no
collectives, each core writes a disjoint output slice.

v2 redesign vs v1 (fp32r, DRAM-spilled):
  - All matmul operands bf16 (1 cyc/row like fp32r, half the SBUF/DMA).
    Everything lives in SBUF; no DRAM spills of qh/kh/ct.
  - Head-pair (j) software pipelining: projections for pair j+1 are
    emitted before attention for pair j, so PE streams projections while
    ACT/DVE chew on exp/mask of the previous pair.
  - Algebraic bias folds: k-bias shifts every score of a q-row equally
    -> softmax-invariant -> dropped.  v-bias passes through attention
    (weights sum to 1) -> folded into b_o on host (b_o' = b_v @ W_o +
    b_o).  1/sqrt(dk) and q-bias folded into W_q/b_q on host.
  - Attention (per head): scoresT[k,q] chunks [128,1024] on 2 psum
    banks; exp on ACT (psum fp32 -> sbuf bf16); mask-multiply on DVE in
    bf16 (2x mode); AV transposed: out[q 128, 65] = mexpT_chunk-tile.T @
    [vh|1], accumulated over 16 k-chunks (65 rows/matmul vs 1024 of the
    d-on-partitions scheme), denominator = column 64.
  - Normalize = DVE reciprocal [128,1] + per-partition tensor_scalar
    multiply (no partition-broadcast DMA round trip).
  - ct [q,d] -> ctT via PE transposes (f32, identity matmul) into a
    borrowed scores-psum tile; single DVE drain [128,1024] per pair.
  - Output projection accumulates in psum, bias added via ones-row
    matmul, result DMA'd to DRAM straight from PSUM.

PSUM budget (8 banks): scores 2x[128,1024]f32 (4) + AV accum
1x[128,1024]f32 (2, 65 of each 128-col block used) + proj 2x[128,512]
(2).  Transposes borrow scores tiles at pair boundaries.
"""

import os
import sys

if "/opt/trn_rl_repo" not in sys.path:
    sys.path.insert(0, "/opt/trn_rl_repo")
os.environ.setdefault("MYCRO_LOCAL_CACHE", "1")

import numpy as np
import ml_dtypes

import concourse.bass as bass
import concourse.bacc as bacc
import concourse.mybir as mybir
import concourse.tile as tile
from concourse.bass import ts

B, S, D, H, DK = 4, 2048, 1024, 16, 64
SQ = S // 2          # q rows per core
P = 128
NC_K = S // P        # 16 k-chunks
NJ = D // P          # 8 feature chunks = head pairs
NQT = SQ // P        # 8 q tiles
N_CORES = 8

F32 = mybir.dt.float32
BF16 = mybir.dt.bfloat16
AF = mybir.ActivationFunctionType


def build_program():
    nc = bacc.Bacc(
        "TRN2",
        target_bir_lowering=False,
        debug=False,
        enable_asserts=False,
    )

    # ---- DRAM I/O (per-core slices; host pre-transposed, bf16) ----
    qT_d = nc.dram_tensor("qT", [D, SQ], BF16, kind="ExternalInput").ap()
    kT_d = nc.dram_tensor("kT", [D, S], BF16, kind="ExternalInput").ap()
    vT_d = nc.dram_tensor("vT", [D, S], BF16, kind="ExternalInput").ap()
    mT_d = nc.dram_tensor("maskT", [S, SQ], BF16, kind="ExternalInput").ap()
    wq_d = nc.dram_tensor("wq", [D, D], BF16, kind="ExternalInput").ap()
    wk_d = nc.dram_tensor("wk", [D, D], BF16, kind="ExternalInput").ap()
    wv_d = nc.dram_tensor("wv", [D, D], BF16, kind="ExternalInput").ap()
    wo_d = nc.dram_tensor("wo", [D, D], BF16, kind="ExternalInput").ap()
    bq_d = nc.dram_tensor("bq", [D], F32, kind="ExternalInput").ap()
    bo_d = nc.dram_tensor("bo", [D], BF16, kind="ExternalInput").ap()
    ones_d = nc.dram_tensor("ones_row", [1, P], BF16, kind="ExternalInput").ap()
    ident_d = nc.dram_tensor("ident", [P, P], F32, kind="ExternalInput").ap()
    out_d = nc.dram_tensor("out", [SQ, D], F32, kind="ExternalOutput").ap()

    with tile.TileContext(nc) as tc:
        _build(nc, tc, qT_d, kT_d, vT_d, mT_d, wq_d, wk_d, wv_d, wo_d,
               bq_d, bo_d, ones_d, ident_d, out_d)

    nc.compile()
    return nc


def _build(nc, tc, qT_d, kT_d, vT_d, mT_d, wq_d, wk_d, wv_d, wo_d,
           bq_d, bo_d, ones_d, ident_d, out_d):
    from contextlib import ExitStack

    with ExitStack() as top:
        # ---------------- persistent SBUF ----------------
        consts = top.enter_context(tc.tile_pool(name="consts", bufs=1))
        ones1 = consts.tile([1, P], BF16, tag="ones1")
        bo_row = consts.tile([1, D], BF16, tag="bo_row")
        ident = consts.tile([P, P], F32, tag="ident")
        bq_sb = consts.tile([P, NJ], F32, tag="bq_sb")

        inp = top.enter_context(tc.tile_pool(name="inp", bufs=1))
        qT = [inp.tile([P, SQ], BF16, tag=f"qT{d}", name=f"qT{d}")
              for d in range(NJ)]
        kT = [inp.tile([P, S], BF16, tag=f"kT{d}", name=f"kT{d}")
              for d in range(NJ)]
        vT = [inp.tile([P, S], BF16, tag=f"vT{d}", name=f"vT{d}")
              for d in range(NJ)]
        maskT = [inp.tile([P, SQ], BF16, tag=f"mT{c}", name=f"mT{c}")
                 for c in range(NC_K)]
        wo_sb = [inp.tile([P, D], BF16, tag=f"wo{d}", name=f"wo{d}")
                 for d in range(NJ)]
        ctT = [inp.tile([P, SQ], BF16, tag=f"ctT{d}", name=f"ctT{d}")
               for d in range(NJ)]

        # DMA order = startup critical path: w-slices for j=0,1 are issued
        # inside emit_proj; here load qT first (q-proj-0 needs all of it),
        # then kT, a few mask tiles, vT, rest of mask, wo, consts.
        nc.sync.dma_start(ones1[:], ones_d)
        nc.sync.dma_start(bq_sb[:], bq_d.rearrange("(j p) -> p j", p=P))
        for d in range(NJ):
            nc.sync.dma_start(qT[d][:], qT_d[ts(d, P), :])
        for d in range(NJ):
            nc.sync.dma_start(kT[d][:], kT_d[ts(d, P), :])
        for c in range(4):
            nc.sync.dma_start(maskT[c][:], mT_d[ts(c, P), :])
        for d in range(NJ):
            nc.sync.dma_start(vT[d][:], vT_d[ts(d, P), :])
        for c in range(4, NC_K):
            nc.sync.dma_start(maskT[c][:], mT_d[ts(c, P), :])
        nc.sync.dma_start(ident[:], ident_d)
        nc.sync.dma_start(bo_row[:], bo_d.rearrange("(o n) -> o n", o=1))
        for d in range(NJ):
            nc.sync.dma_start(wo_sb[d][:], wo_d[ts(d, P), :])

        # ---------------- ring pools (head-pair pipelined) ----------------
        wsl = top.enter_context(tc.tile_pool(name="wsl", bufs=2))
        qh_p = top.enter_context(tc.tile_pool(name="qh", bufs=2))
        kh_p = top.enter_context(tc.tile_pool(name="kh", bufs=2))
        vh_p = top.enter_context(tc.tile_pool(name="vh", bufs=2))
        ctp_p = top.enter_context(tc.tile_pool(name="ctp", bufs=2))
        et_p = top.enter_context(tc.tile_pool(name="et", bufs=2))
        mt_p = top.enter_context(tc.tile_pool(name="mt", bufs=3))
        rc_p = top.enter_context(tc.tile_pool(name="rc", bufs=4))

        ps_scr = top.enter_context(
            tc.tile_pool(name="ps_scr", bufs=2, space="PSUM"))
        ps_av = top.enter_context(
            tc.tile_pool(name="ps_av", bufs=1, space="PSUM"))
        ps_pr = top.enter_context(
            tc.tile_pool(name="ps_pr", bufs=2, space="PSUM"))

        # W slice view: DRAM W[1024, 1024] rows (dc p), cols j*128+f
        # -> SBUF [p, (dc f)] = [128, 1024]
        def w_slice_ap(w_d, j):
            return w_d.rearrange("(dc p) f -> p (dc f)", p=P)[
                :, :].rearrange("p (dc f) -> p dc f", f=D)[
                :, :, ts(j, P)].rearrange("p dc f -> p (dc f)")

        state = {}

        def emit_proj(j):
            """Projections for head pair j -> qh_j, kh_j, vh_j."""
            wq_t = wsl.tile([P, D], BF16, tag="wq_sl")
            wk_t = wsl.tile([P, D], BF16, tag="wk_sl")
            wv_t = wsl.tile([P, D], BF16, tag="wv_sl")
            nc.sync.dma_start(wq_t[:], w_slice_ap(wq_d, j))
            nc.sync.dma_start(wk_t[:], w_slice_ap(wk_d, j))
            nc.sync.dma_start(wv_t[:], w_slice_ap(wv_d, j))

            qh = qh_p.tile([P, SQ], BF16, tag="qh")
            kh = kh_p.tile([P, S], BF16, tag="kh")
            vh = [vh_p.tile([P, 2 * (DK + 1)], BF16, tag=f"vh{c}",
                            name=f"vh{c}_{j}") for c in range(NC_K)]

            # q-proj: out[feat 128, q 512] x2, contraction over 8 d-chunks
            for half in range(SQ // 512):
                pq = ps_pr.tile([P, 512], F32, tag="ppr")
                for d in range(NJ):
                    nc.tensor.matmul(
                        pq[:], wq_t[:, ts(d, P)], qT[d][:, ts(half, 512)],
                        start=(d == 0), stop=(d == NJ - 1))
                nc.vector.tensor_scalar_add(
                    qh[:, ts(half, 512)], pq[:], bq_sb[:, ts(j, 1)])
            # k-proj: out[feat 128, k 512] x4 (no bias: softmax-invariant)
            for half in range(S // 512):
                pk = ps_pr.tile([P, 512], F32, tag="ppr")
                for d in range(NJ):
                    nc.tensor.matmul(
                        pk[:], wk_t[:, ts(d, P)], kT[d][:, ts(half, 512)],
                        start=(d == 0), stop=(d == NJ - 1))
                nc.vector.tensor_copy(kh[:, ts(half, 512)], pk[:])
            # v-proj: out[kpos 128, feat 128] per k-chunk, 4 chunks per psum
            # (v-bias folded into b_o on host)
            for g in range(NC_K // 4):
                pv = ps_pr.tile([P, 512], F32, tag="ppr")
                for cc in range(4):
                    c = g * 4 + cc
                    for d in range(NJ):
                        nc.tensor.matmul(
                            pv[:, ts(cc, P)], vT[d][:, ts(c, P)],
                            wv_t[:, ts(d, P)],
                            start=(d == 0), stop=(d == NJ - 1))
                for cc in range(4):
                    c = g * 4 + cc
                    dst = vh[c].rearrange("p (h w) -> p h w", w=DK + 1)
                    src = pv[:, ts(cc, P)].rearrange("p (h w) -> p h w", w=DK)
                    nc.vector.tensor_copy(dst[:, :, 0:DK], src[:])
                    nc.gpsimd.memset(dst[:, :, DK:DK + 1], 1.0)
            state[j] = (qh, kh, vh)

        def emit_attn(j):
            qh, kh, vh = state.pop(j)
            ctp = [ctp_p.tile([P, P], F32, tag=f"ctp{qt}",
                              name=f"ctp{qt}_{j}") for qt in range(NQT)]
            for hh in range(2):
                pav = ps_av.tile([P, SQ], F32, tag="pav")
                for c in range(NC_K):
                    pscr = ps_scr.tile([P, SQ], F32, tag="pscr")
                    for half in range(SQ // 512):
                        nc.tensor.matmul(
                            pscr[:, ts(half, 512)],
                            kh[ts(hh, DK), ts(c, P)],
                            qh[ts(hh, DK), ts(half, 512)],
                            start=True, stop=True)
                    et = et_p.tile([P, SQ], BF16, tag="et")
                    nc.scalar.activation(et[:], pscr[:], AF.Exp)
                    mt = mt_p.tile([P, SQ], BF16, tag="mt")
                    nc.vector.tensor_mul(mt[:], et[:], maskT[c][:])
                    for qt in range(NQT):
                        nc.tensor.matmul(
                            pav[:, qt * P: qt * P + DK + 1],
                            mt[:, ts(qt, P)],
                            vh[c][:, hh * (DK + 1): (hh + 1) * (DK + 1)],
                            start=(c == 0), stop=(c == NC_K - 1))
                for qt in range(NQT):
                    rc = rc_p.tile([P, 1], F32, tag="rc")
                    nc.vector.reciprocal(
                        rc[:], pav[:, qt * P + DK: qt * P + DK + 1])
                    nc.vector.tensor_scalar_mul(
                        ctp[qt][:, ts(hh, DK)],
                        pav[:, qt * P: qt * P + DK], rc[:])
            state[("ct", j)] = ctp

        def emit_transp(j):
            ctp = state.pop(("ct", j))
            pt = ps_scr.tile([P, SQ], F32, tag="pscr")
            for qt in range(NQT):
                nc.tensor.transpose(pt[:, ts(qt, P)], ctp[qt][:], ident[:])
            nc.vector.tensor_copy(ctT[j][:], pt[:])

        def emit_oproj():
            ps_o = ps_scr  # scores psum free by now; reuse the pool
            for qt in range(NQT):
                po = ps_o.tile([P, SQ], F32, tag="pscr")
                for half in range(D // 512):
                    for d in range(NJ):
                        nc.tensor.matmul(
                            po[:, ts(half, 512)],
                            ctT[d][:, ts(qt, P)],
                            wo_sb[d][:, ts(half, 512)],
                            start=(d == 0), stop=False)
                    nc.tensor.matmul(
                        po[:, ts(half, 512)], ones1[:],
                        bo_row[:, ts(half, 512)],
                        start=False, stop=True)
                nc.sync.dma_start(out_d[ts(qt, P), :], po[:])

        emit_proj(0)
        for j in range(NJ):
            if j + 1 < NJ:
                emit_proj(j + 1)
            emit_attn(j)
            emit_transp(j)
        emit_oproj()


def make_in_maps(q, k, v, att_mask):
    """Build the 8 per-core input dicts (bf16, pre-transposed)."""
    bf = ml_dtypes.bfloat16
    q = np.asarray(q, dtype=np.float32)
    k = np.asarray(k, dtype=np.float32)
    v = np.asarray(v, dtype=np.float32)
    att_mask = np.asarray(att_mask)
    kT_b = [np.ascontiguousarray(k[b].T).astype(bf) for b in range(B)]
    vT_b = [np.ascontiguousarray(v[b].T).astype(bf) for b in range(B)]
    in_maps = []
    for c in range(N_CORES):
        b, half = divmod(c, 2)
        qs = slice(half * SQ, (half + 1) * SQ)
        in_maps.append({
            "qT": np.ascontiguousarray(q[b, qs, :].T).astype(bf),
            "kT": kT_b[b],
            "vT": vT_b[b],
            "maskT": np.ascontiguousarray(att_mask[b, qs, :].T).astype(bf),
        })
    return in_maps


def make_weights(W_q, b_q, W_k, b_k, W_v, b_v, W_o, b_o):
    bf = ml_dtypes.bfloat16
    W_q = np.asarray(W_q, np.float32)
    W_k = np.asarray(W_k, np.float32)
    W_v = np.asarray(W_v, np.float32)
    W_o = np.asarray(W_o, np.float32)
    b_q = np.asarray(b_q, np.float32)
    b_v = np.asarray(b_v, np.float32)
    b_o = np.asarray(b_o, np.float32)
    scale = 1.0 / np.sqrt(DK)
    # k-bias: adds a per-q constant to every score of a row -> softmax
    # invariant -> dropped.  v-bias: attention weights sum to 1 -> passes
    # through -> fold b_v @ W_o into b_o.
    bo_eff = b_v @ W_o + b_o
    return {
        "wq": (W_q * scale).astype(bf),
        "wk": W_k.astype(bf),
        "wv": W_v.astype(bf),
        "wo": W_o.astype(bf),
        "bq": (b_q * scale).astype(np.float32),
        "bo": bo_eff.astype(bf),
        "ones_row": np.ones((1, P), bf),
        "ident": np.eye(P, dtype=np.float32),
    }


_PROG = None


def _get_program():
    global _PROG
    if _PROG is None:
        _PROG = build_program()
    return _PROG


def kernel(q, k, v, att_mask, W_q, b_q, W_k, b_k, W_v, b_v, W_o, b_o,
           **_ignored):
    from concourse.bass_utils import run_bass_kernel_spmd

    nc = _get_program()
    weights = make_weights(W_q, b_q, W_k, b_k, W_v, b_v, W_o, b_o)
    in_maps = [dict(m, **weights) for m in make_in_maps(q, k, v, att_mask)]
    res = run_bass_kernel_spmd(nc, in_maps, core_ids=list(range(N_CORES)))
    out = np.empty((B, S, D), dtype=np.float32)
    for c in range(N_CORES):
        b, half = divmod(c, 2)
        out[b, half * SQ:(half + 1) * SQ, :] = res.results[c]["out"]
    return out
